# revision 26
# baseline (speedup 1.0000x reference)
"""Trainium2 Bass kernel for nn_MultiClassAttentionHead.

Computation (per sample b):
  global[b]  = class_token[b] @ gc_w.T + gc_b                      (C,)
  att[b]     = sigmoid(attn_w @ patch[b].T + attn_b[:, None])      (C, S)
  ts[b, s]   = sum_d patch[b, s, d]                                (S,)
  A2[b, c]   = sum_s att[b, c, s] * ts[b, s] / (S*D)
  out[b]     = global[b] + lam * A2[b]

Sharding: data-parallel over batch B=64 across 8 cores (8 samples each),
weights replicated; no cross-device communication (host gathers outputs).

Per-core kernel strategy (v4):
  * Whole pipeline in bf16 (cast during the SWDGE DMA load); the 2e-2
    tolerance leaves ~10x headroom over bf16 rounding of the dominant
    global term.
  * 8 samples as 4 pairs of 1152 rows.  DMA layout "(p n) d": partition
    p holds rows 9p..9p+8 of the pair, so each partition's source is ONE
    contiguous DRAM segment per n-group load -> 128 descriptors per
    load, keeping the Q7 SWDGE descriptor generator far ahead of SDMA.
    Sample boundary falls exactly at partition 64 for every n.
    Pair 0's three n-group loads are emitted before everything else on
    the gpsimd queue so HBM streaming starts as early as possible; all
    nat tiles are resident (bufs=PAIRS) so the stream never WAR-stalls.
  * Transposes: per (pair, n) six 128x128 PE transposes (is_transpose,
    bf16 PSUM output = 1 bank) -> one [128,768] PSUM->SBUF copy,
    alternating DVE/ACT.
  * einsum1 (A-layout): out[s-chunk, 201] = sum_dc P^T-chunk^T @ waug
    where waug = [W^T | lam*ones/(S*D)]; column 200 gives lam-scaled
    token sums for free.  attn_b added on DVE before the sigmoid.
  * sigmoid on ACT -> attT (s, c) bf16.
  * einsum2: all pairs accumulate into ONE [8, C] PSUM tile; pair p's
    stationary is a [128, 8] zero-masked ts-matrix (col 2p = ts on
    partitions 0..63 = sample 0, col 2p+1 = partitions 64..127 =
    sample 1) -> one N=200 matmul per chunk, base-partition-0 output.
  * Global-scores setup emitted mid-loop as low-priority PE filler;
    waug setup ping-pongs PSUM banks so PE never waits on copy drains.
"""

import sys

if "/opt/trn_rl_repo" not in sys.path:
    sys.path.insert(0, "/opt/trn_rl_repo")

import numpy as np

import concourse.bass as bass
import concourse.tile as tile
from concourse import bacc, mybir
from concourse.bass_utils import run_bass_kernel_spmd
from concourse.masks import make_identity

B, S, D, C = 64, 576, 768, 200
NCORES = 8
BPC = B // NCORES          # samples per core
PAIRS = BPC // 2           # sample pairs per core
N_PP = (2 * S) // 128      # 9 n-slots per pair
DC = D // 128              # 6 d-chunks
INV_SD = 1.0 / float(S * D)

F32 = mybir.dt.float32
BF16 = mybir.dt.bfloat16
AF = mybir.ActivationFunctionType

_COMPILED = None


def _build():
    nc = bacc.Bacc("TRN2", target_bir_lowering=False, debug=False,
                   num_devices=NCORES)

    pt = nc.dram_tensor("pt", [BPC, S, D], F32, kind="ExternalInput")
    ct = nc.dram_tensor("ct", [BPC, D], F32, kind="ExternalInput")
    aw = nc.dram_tensor("aw", [C, D], F32, kind="ExternalInput")
    ab = nc.dram_tensor("ab", [C], F32, kind="ExternalInput")
    gw = nc.dram_tensor("gw", [C, D], F32, kind="ExternalInput")
    gb = nc.dram_tensor("gb", [C], F32, kind="ExternalInput")
    lam = nc.dram_tensor("lam", [1], F32, kind="ExternalInput")
    out = nc.dram_tensor("out", [BPC, C], F32, kind="ExternalOutput")

    with tile.TileContext(nc) as tc:
        with (
            tc.tile_pool(name="const", bufs=1) as cpool,
            tc.tile_pool(name="nat", bufs=PAIRS) as natpool,
            tc.tile_pool(name="ptr", bufs=2) as ptrpool,
            tc.tile_pool(name="att", bufs=2) as attpool,
            tc.tile_pool(name="ps_tr", bufs=3, space="PSUM") as trpsum,
            tc.tile_pool(name="ps_l", bufs=3, space="PSUM") as lpsum,
            tc.tile_pool(name="ps_s", bufs=1, space="PSUM") as spsum,
            tc.tile_pool(name="ps_gs", bufs=1, space="PSUM") as gspsum,
        ):
            # ---- attention weights first on the SWDGE queue (cast-in-DMA
            # straight to bf16: lands ~11us, half the HBM bytes, no
            # engine cast), then pair-0 patch loads ----
            w_b_a = cpool.tile([128, D], BF16)
            w_b_b = cpool.tile([C - 128, D], BF16)
            nc.gpsimd.dma_start(w_b_a[:], aw[0:128, :])
            nc.gpsimd.dma_start(w_b_b[:], aw[128:C, :])

            def load_pair(p, nat):
                pair = pt[2 * p:2 * p + 2, :, :] \
                    .rearrange("b s d -> (b s) d") \
                    .rearrange("(p n) d -> p n d", p=128)
                for g in range(3):
                    nc.gpsimd.dma_start(nat[:, 3 * g:3 * g + 3, :],
                                        pair[:, 3 * g:3 * g + 3, :])

            nats = {p: natpool.tile([128, N_PP, D], BF16, tag="nat",
                                    name=f"nat{p}") for p in range(PAIRS)}
            load_pair(0, nats[0])

            # ---- tiny gpsimd constants (needed from ~11us), then the
            # remaining pair loads (SDMA is busy with pair 0 anyway) ----
            ident_b = cpool.tile([128, 128], BF16)
            make_identity(nc, ident_b[:])
            ones_row_f = cpool.tile([1, 128], F32)
            nc.gpsimd.memset(ones_row_f[:], 1.0)
            for p in range(1, PAIRS):
                load_pair(p, nats[p])

            # ---- remaining loads: global weights on sync (HWDGE, f32,
            # needed only by k==12); small scalars on the scalar queue ----
            g_f_a = cpool.tile([128, D], F32)
            g_f_b = cpool.tile([C - 128, D], F32)
            nc.sync.dma_start(g_f_a[:], gw[0:128, :])
            nc.sync.dma_start(g_f_b[:], gw[128:C, :])
            ct_f = cpool.tile([BPC, D], F32)
            nc.sync.dma_start(ct_f[:], ct[:])

            lam_one = cpool.tile([1, 1], F32)
            nc.scalar.dma_start(lam_one[:], lam[:].rearrange("(a c) -> a c", a=1))
            ab_f = cpool.tile([1, C], F32)
            nc.scalar.dma_start(ab_f[:], ab[:].rearrange("(a c) -> a c", a=1))
            gb_row = cpool.tile([1, C], F32)
            nc.scalar.dma_start(gb_row[:], gb[:].rearrange("(a c) -> a c", a=1))

            # ---- broadcasts (PE outer products off small scalars) ----
            ps_lam = spsum.tile([128, 1], F32, tag="sp")
            nc.tensor.matmul(ps_lam[:], ones_row_f[:], lam_one[:],
                             start=True, stop=True)
            lam_sb = cpool.tile([128, 1], F32)
            nc.vector.tensor_copy(lam_sb[:], ps_lam[:])

            ps_bb = gspsum.tile([128, C], F32, tag="gsp")
            nc.tensor.matmul(ps_bb[:], ones_row_f[:], ab_f[:],
                             start=True, stop=True)
            bias_bc = cpool.tile([128, C], F32)
            nc.vector.tensor_copy(bias_bc[:], ps_bb[:])

            # ---- attn_w -> waug (128, DC, C+1) bf16: [W^T | lam/(S*D)];
            # PSUM ping-pongs between the two 1-buf setup pools, copies
            # alternate DVE/ACT so neither queue serializes the chain ----
            waug = cpool.tile([128, DC, C + 1], BF16)
            for dc in range(DC):
                pool, tag = (spsum, "sp") if dc % 2 == 0 else (gspsum, "gsp")
                ps_w = pool.tile([128, C], F32, tag=tag, name=f"ps_w{dc}")
                nc.tensor.matmul(ps_w[:, 0:128],
                                 w_b_a[:, dc * 128:(dc + 1) * 128],
                                 ident_b[:], start=True, stop=True)
                nc.tensor.matmul(ps_w[:, 128:C],
                                 w_b_b[:, dc * 128:(dc + 1) * 128],
                                 ident_b[0:C - 128, 0:C - 128],
                                 start=True, stop=True)
                if dc % 2 == 0:
                    nc.vector.tensor_copy(waug[:, dc, 0:C], ps_w[:])
                else:
                    nc.scalar.copy(waug[:, dc, 0:C], ps_w[:])
                nc.scalar.activation(waug[:, dc, C:C + 1], lam_sb[:],
                                     AF.Copy, scale=INV_SD)

            # ---- global-scores setup (emitted mid-loop as PE filler) ----
            gs_sb = cpool.tile([BPC, C], F32)

            def emit_gs_setup():
                g_b_a = cpool.tile([128, D], BF16)
                g_b_b = cpool.tile([C - 128, D], BF16)
                nc.vector.tensor_copy(g_b_a[:], g_f_a[:])
                nc.scalar.copy(g_b_b[:], g_f_b[:])
                ct_b = cpool.tile([BPC, D], BF16)
                nc.vector.tensor_copy(ct_b[:], ct_f[:])

                gwT = cpool.tile([128, DC, C], BF16)
                for dc in range(DC):
                    ps_g = gspsum.tile([128, C], F32, tag="gsp",
                                       name=f"ps_g{dc}")
                    nc.tensor.matmul(ps_g[:, 0:128],
                                     g_b_a[:, dc * 128:(dc + 1) * 128],
                                     ident_b[:], start=True, stop=True)
                    nc.tensor.matmul(ps_g[:, 128:C],
                                     g_b_b[:, dc * 128:(dc + 1) * 128],
                                     ident_b[0:C - 128, 0:C - 128],
                                     start=True, stop=True)
                    if dc % 2 == 0:
                        nc.scalar.copy(gwT[:, dc, :], ps_g[:])
                    else:
                        nc.vector.tensor_copy(gwT[:, dc, :], ps_g[:])

                ctT = cpool.tile([128, DC, BPC], BF16)
                for dc in range(DC):
                    ps_c = gspsum.tile([128, BPC], F32, tag="gsp",
                                       name=f"ps_c{dc}")
                    nc.tensor.matmul(ps_c[:],
                                     ct_b[:, dc * 128:(dc + 1) * 128],
                                     ident_b[0:BPC, 0:BPC],
                                     start=True, stop=True)
                    nc.vector.tensor_copy(ctT[:, dc, :], ps_c[:])

                ps_gs = gspsum.tile([BPC, C], F32, tag="gsp")
                nc.tensor.matmul(ps_gs[:], ones_row_f[0:1, 0:BPC], gb_row[:],
                                 start=True, stop=False)
                for dc in range(DC):
                    nc.tensor.matmul(ps_gs[:], ctT[:, dc, :], gwT[:, dc, :],
                                     start=False, stop=(dc == DC - 1))
                nc.vector.tensor_copy(gs_sb[:], ps_gs[:])

            # ---------------- main loop over sample pairs ----------------
            outsb = cpool.tile([BPC, C], F32)
            ptrs, attTs, tsms = {}, {}, {}
            ps_a2_box = []

            def emit_tr(p, n):
                if n == 0:
                    ptrs[p] = ptrpool.tile([128, N_PP, DC, 128], BF16,
                                           tag="ptr", name=f"ptr{p}")
                    attTs[p] = attpool.tile([128, N_PP, C], BF16,
                                            tag="attT", name=f"attT{p}")
                    tsm = attpool.tile([128, BPC, N_PP], BF16, tag="tsm",
                                       name=f"tsm{p}")
                    nc.vector.memset(tsm[:], 0.0)
                    tsms[p] = tsm
                nat = nats[p]
                ps_tr = trpsum.tile([128, D], BF16, tag="tr",
                                    name=f"tr_{p}_{n}")
                for j in range(DC):
                    nc.tensor.transpose(
                        ps_tr[:, j * 128:(j + 1) * 128],
                        nat[:, n, j * 128:(j + 1) * 128],
                        ident_b[:])
                if n % 2 == 0:
                    nc.vector.tensor_copy(ptrs[p][:, n, :, :], ps_tr[:])
                else:
                    nc.scalar.copy(ptrs[p][:, n, :, :], ps_tr[:])

            def emit_e1(p, n):
                ps_l = lpsum.tile([128, C + 1], F32, tag="l")
                for dc in range(DC):
                    nc.tensor.matmul(ps_l[:], ptrs[p][:, n, dc, :],
                                     waug[:, dc, :],
                                     start=(dc == 0), stop=(dc == DC - 1))
                nc.vector.tensor_add(ps_l[:, 0:C], ps_l[:, 0:C], bias_bc[:])
                nc.scalar.activation(attTs[p][:, n, :], ps_l[:, 0:C],
                                     AF.Sigmoid)
                # lam-scaled token sums -> masked ts-matrix columns
                tsm = tsms[p]
                nc.vector.tensor_copy(tsm[0:64, 2 * p, n:n + 1],
                                      ps_l[0:64, C:C + 1])
                nc.vector.tensor_copy(tsm[64:128, 2 * p + 1, n:n + 1],
                                      ps_l[64:128, C:C + 1])

            def emit_e2(p):
                if not ps_a2_box:
                    ps_a2_box.append(spsum.tile([BPC, C], F32, tag="sp",
                                                name="ps_a2"))
                ps_a2 = ps_a2_box[0]
                attT, tsm = attTs.pop(p), tsms.pop(p)
                del ptrs[p]
                for n in range(N_PP):
                    nc.tensor.matmul(ps_a2[:],
                                     tsm[:, :, n],
                                     attT[:, n, :],
                                     start=(p == 0 and n == 0),
                                     stop=(p == PAIRS - 1 and n == N_PP - 1))

            TOT = PAIRS * N_PP
            for k in range(TOT):
                p, n = divmod(k, N_PP)
                emit_tr(p, n)
                if k == 12:
                    emit_gs_setup()
                if k >= 3:
                    emit_e1(*divmod(k - 3, N_PP))
                if n == 3 and p >= 1:
                    emit_e2(p - 1)
            emit_e1(PAIRS - 1, N_PP - 3)
            emit_e1(PAIRS - 1, N_PP - 2)
            emit_e1(PAIRS - 1, N_PP - 1)
            emit_e2(PAIRS - 1)

            # ---------------- final combine + store ----------------
            nc.vector.tensor_add(outsb[:], ps_a2_box[0][:], gs_sb[:])
            nc.sync.dma_start(out[:], outsb[:])

    nc.compile()
    return nc


def _get_compiled():
    global _COMPILED
    if _COMPILED is None:
        _COMPILED = _build()
    return _COMPILED


def kernel(patch_tokens, class_token, attn_w, attn_b, gc_w, gc_b, lam,
           **_ignored):
    nc = _get_compiled()
    patch_tokens = np.ascontiguousarray(patch_tokens, dtype=np.float32)
    class_token = np.ascontiguousarray(class_token, dtype=np.float32)
    attn_w = np.ascontiguousarray(attn_w, dtype=np.float32)
    attn_b = np.ascontiguousarray(attn_b, dtype=np.float32)
    gc_w = np.ascontiguousarray(gc_w, dtype=np.float32)
    gc_b = np.ascontiguousarray(gc_b, dtype=np.float32)
    lam = np.ascontiguousarray(lam, dtype=np.float32)

    in_maps = []
    for i in range(NCORES):
        sl = slice(i * BPC, (i + 1) * BPC)
        in_maps.append({
            "pt": patch_tokens[sl],
            "ct": class_token[sl],
            "aw": attn_w,
            "ab": attn_b,
            "gw": gc_w,
            "gb": gc_b,
            "lam": lam,
        })
    res = run_bass_kernel_spmd(nc, in_maps, core_ids=list(range(NCORES)))
    return np.concatenate([res.results[i]["out"] for i in range(NCORES)],
                          axis=0)


# revision 27
# speedup vs baseline: 1.0040x; 1.0040x over previous
"""Trainium2 Bass kernel for nn_MultiClassAttentionHead.

Computation (per sample b):
  global[b]  = class_token[b] @ gc_w.T + gc_b                      (C,)
  att[b]     = sigmoid(attn_w @ patch[b].T + attn_b[:, None])      (C, S)
  ts[b, s]   = sum_d patch[b, s, d]                                (S,)
  A2[b, c]   = sum_s att[b, c, s] * ts[b, s] / (S*D)
  out[b]     = global[b] + lam * A2[b]

Sharding: data-parallel over batch B=64 across 8 cores (8 samples each),
weights replicated; no cross-device communication (host gathers outputs).

Per-core kernel strategy (v4):
  * Whole pipeline in bf16 (cast during the SWDGE DMA load); the 2e-2
    tolerance leaves ~10x headroom over bf16 rounding of the dominant
    global term.
  * 8 samples as 4 pairs of 1152 rows.  DMA layout "(p n) d": partition
    p holds rows 9p..9p+8 of the pair, so each partition's source is ONE
    contiguous DRAM segment per n-group load -> 128 descriptors per
    load, keeping the Q7 SWDGE descriptor generator far ahead of SDMA.
    Sample boundary falls exactly at partition 64 for every n.
    Pair 0's three n-group loads are emitted before everything else on
    the gpsimd queue so HBM streaming starts as early as possible; all
    nat tiles are resident (bufs=PAIRS) so the stream never WAR-stalls.
  * Transposes: per (pair, n) six 128x128 PE transposes (is_transpose,
    bf16 PSUM output = 1 bank) -> one [128,768] PSUM->SBUF copy,
    alternating DVE/ACT.
  * einsum1 (A-layout): out[s-chunk, 201] = sum_dc P^T-chunk^T @ waug
    where waug = [W^T | lam*ones/(S*D)]; column 200 gives lam-scaled
    token sums for free.  attn_b added on DVE before the sigmoid.
  * sigmoid on ACT -> attT (s, c) bf16.
  * einsum2: all pairs accumulate into ONE [8, C] PSUM tile; pair p's
    stationary is a [128, 8] zero-masked ts-matrix (col 2p = ts on
    partitions 0..63 = sample 0, col 2p+1 = partitions 64..127 =
    sample 1) -> one N=200 matmul per chunk, base-partition-0 output.
  * Global-scores setup emitted mid-loop as low-priority PE filler;
    waug setup ping-pongs PSUM banks so PE never waits on copy drains.
"""

import sys

if "/opt/trn_rl_repo" not in sys.path:
    sys.path.insert(0, "/opt/trn_rl_repo")

import numpy as np

import concourse.bass as bass
import concourse.tile as tile
from concourse import bacc, mybir
from concourse.bass_utils import run_bass_kernel_spmd
from concourse.masks import make_identity

B, S, D, C = 64, 576, 768, 200
NCORES = 8
BPC = B // NCORES          # samples per core
PAIRS = BPC // 2           # sample pairs per core
N_PP = (2 * S) // 128      # 9 n-slots per pair
DC = D // 128              # 6 d-chunks
INV_SD = 1.0 / float(S * D)

F32 = mybir.dt.float32
BF16 = mybir.dt.bfloat16
AF = mybir.ActivationFunctionType

_COMPILED = None


def _build():
    nc = bacc.Bacc("TRN2", target_bir_lowering=False, debug=False,
                   num_devices=NCORES)

    pt = nc.dram_tensor("pt", [BPC, S, D], F32, kind="ExternalInput")
    ct = nc.dram_tensor("ct", [BPC, D], F32, kind="ExternalInput")
    aw = nc.dram_tensor("aw", [C, D], F32, kind="ExternalInput")
    ab = nc.dram_tensor("ab", [C], F32, kind="ExternalInput")
    gw = nc.dram_tensor("gw", [C, D], F32, kind="ExternalInput")
    gb = nc.dram_tensor("gb", [C], F32, kind="ExternalInput")
    lam = nc.dram_tensor("lam", [1], F32, kind="ExternalInput")
    out = nc.dram_tensor("out", [BPC, C], F32, kind="ExternalOutput")

    with tile.TileContext(nc) as tc:
        with (
            tc.tile_pool(name="const", bufs=1) as cpool,
            tc.tile_pool(name="nat", bufs=PAIRS) as natpool,
            tc.tile_pool(name="ptr", bufs=2) as ptrpool,
            tc.tile_pool(name="att", bufs=2) as attpool,
            tc.tile_pool(name="ps_tr", bufs=2, space="PSUM") as trpsum,
            tc.tile_pool(name="ps_l", bufs=3, space="PSUM") as lpsum,
            tc.tile_pool(name="ps_s", bufs=1, space="PSUM") as spsum,
            tc.tile_pool(name="ps_gs", bufs=1, space="PSUM") as gspsum,
        ):
            # ---- attention weights first on the SWDGE queue (cast-in-DMA
            # straight to bf16: lands ~11us, half the HBM bytes, no
            # engine cast), then pair-0 patch loads ----
            w_b_a = cpool.tile([128, D], BF16)
            w_b_b = cpool.tile([C - 128, D], BF16)
            nc.gpsimd.dma_start(w_b_a[:], aw[0:128, :])
            nc.gpsimd.dma_start(w_b_b[:], aw[128:C, :])

            def load_pair(p, nat):
                pair = pt[2 * p:2 * p + 2, :, :] \
                    .rearrange("b s d -> (b s) d") \
                    .rearrange("(p n) d -> p n d", p=128)
                for g in range(3):
                    nc.gpsimd.dma_start(nat[:, 3 * g:3 * g + 3, :],
                                        pair[:, 3 * g:3 * g + 3, :])

            nats = {p: natpool.tile([128, N_PP, D], BF16, tag="nat",
                                    name=f"nat{p}") for p in range(PAIRS)}
            load_pair(0, nats[0])

            # ---- tiny gpsimd constants (needed from ~11us), then the
            # remaining pair loads (SDMA is busy with pair 0 anyway) ----
            ident_b = cpool.tile([128, 128], BF16)
            make_identity(nc, ident_b[:])
            ones_row_f = cpool.tile([1, 128], F32)
            nc.gpsimd.memset(ones_row_f[:], 1.0)
            for p in range(1, PAIRS):
                load_pair(p, nats[p])

            # ---- remaining loads: global weights on sync (HWDGE, f32,
            # needed only by k==12); small scalars on the scalar queue ----
            g_f_a = cpool.tile([128, D], F32)
            g_f_b = cpool.tile([C - 128, D], F32)
            nc.sync.dma_start(g_f_a[:], gw[0:128, :])
            nc.sync.dma_start(g_f_b[:], gw[128:C, :])
            ct_f = cpool.tile([BPC, D], F32)
            nc.sync.dma_start(ct_f[:], ct[:])

            lam_one = cpool.tile([1, 1], F32)
            nc.scalar.dma_start(lam_one[:], lam[:].rearrange("(a c) -> a c", a=1))
            ab_f = cpool.tile([1, C], F32)
            nc.scalar.dma_start(ab_f[:], ab[:].rearrange("(a c) -> a c", a=1))
            gb_row = cpool.tile([1, C], F32)
            nc.scalar.dma_start(gb_row[:], gb[:].rearrange("(a c) -> a c", a=1))

            # ---- broadcasts (PE outer products off small scalars) ----
            ps_lam = spsum.tile([128, 1], F32, tag="sp")
            nc.tensor.matmul(ps_lam[:], ones_row_f[:], lam_one[:],
                             start=True, stop=True)
            lam_sb = cpool.tile([128, 1], F32)
            nc.vector.tensor_copy(lam_sb[:], ps_lam[:])

            ps_bb = gspsum.tile([128, C], F32, tag="gsp")
            nc.tensor.matmul(ps_bb[:], ones_row_f[:], ab_f[:],
                             start=True, stop=True)
            bias_bc = cpool.tile([128, C], F32)
            nc.vector.tensor_copy(bias_bc[:], ps_bb[:])

            # ---- attn_w -> waug (128, DC, C+1) bf16: [W^T | lam/(S*D)];
            # PSUM ping-pongs between the two 1-buf setup pools, copies
            # alternate DVE/ACT so neither queue serializes the chain ----
            waug = cpool.tile([128, DC, C + 1], BF16)
            for dc in range(DC):
                pool, tag = (spsum, "sp") if dc % 2 == 0 else (gspsum, "gsp")
                ps_w = pool.tile([128, C], F32, tag=tag, name=f"ps_w{dc}")
                nc.tensor.matmul(ps_w[:, 0:128],
                                 w_b_a[:, dc * 128:(dc + 1) * 128],
                                 ident_b[:], start=True, stop=True)
                nc.tensor.matmul(ps_w[:, 128:C],
                                 w_b_b[:, dc * 128:(dc + 1) * 128],
                                 ident_b[0:C - 128, 0:C - 128],
                                 start=True, stop=True)
                if dc % 2 == 0:
                    nc.vector.tensor_copy(waug[:, dc, 0:C], ps_w[:])
                else:
                    nc.scalar.copy(waug[:, dc, 0:C], ps_w[:])
                nc.scalar.activation(waug[:, dc, C:C + 1], lam_sb[:],
                                     AF.Copy, scale=INV_SD)

            # ---- global-scores setup (emitted mid-loop as PE filler) ----
            gs_sb = cpool.tile([BPC, C], F32)

            def emit_gs_setup():
                g_b_a = cpool.tile([128, D], BF16)
                g_b_b = cpool.tile([C - 128, D], BF16)
                nc.vector.tensor_copy(g_b_a[:], g_f_a[:])
                nc.scalar.copy(g_b_b[:], g_f_b[:])
                ct_b = cpool.tile([BPC, D], BF16)
                nc.vector.tensor_copy(ct_b[:], ct_f[:])

                gwT = cpool.tile([128, DC, C], BF16)
                for dc in range(DC):
                    ps_g = gspsum.tile([128, C], F32, tag="gsp",
                                       name=f"ps_g{dc}")
                    nc.tensor.matmul(ps_g[:, 0:128],
                                     g_b_a[:, dc * 128:(dc + 1) * 128],
                                     ident_b[:], start=True, stop=True)
                    nc.tensor.matmul(ps_g[:, 128:C],
                                     g_b_b[:, dc * 128:(dc + 1) * 128],
                                     ident_b[0:C - 128, 0:C - 128],
                                     start=True, stop=True)
                    if dc % 2 == 0:
                        nc.scalar.copy(gwT[:, dc, :], ps_g[:])
                    else:
                        nc.vector.tensor_copy(gwT[:, dc, :], ps_g[:])

                ctT = cpool.tile([128, DC, BPC], BF16)
                for dc in range(DC):
                    ps_c = gspsum.tile([128, BPC], F32, tag="gsp",
                                       name=f"ps_c{dc}")
                    nc.tensor.matmul(ps_c[:],
                                     ct_b[:, dc * 128:(dc + 1) * 128],
                                     ident_b[0:BPC, 0:BPC],
                                     start=True, stop=True)
                    nc.vector.tensor_copy(ctT[:, dc, :], ps_c[:])

                ps_gs = gspsum.tile([BPC, C], F32, tag="gsp")
                nc.tensor.matmul(ps_gs[:], ones_row_f[0:1, 0:BPC], gb_row[:],
                                 start=True, stop=False)
                for dc in range(DC):
                    nc.tensor.matmul(ps_gs[:], ctT[:, dc, :], gwT[:, dc, :],
                                     start=False, stop=(dc == DC - 1))
                nc.vector.tensor_copy(gs_sb[:], ps_gs[:])

            # ---------------- main loop over sample pairs ----------------
            outsb = cpool.tile([BPC, C], F32)
            ptrs, attTs, tsms = {}, {}, {}
            ps_a2_box = []

            def emit_tr(p, n):
                if n == 0:
                    ptrs[p] = ptrpool.tile([128, N_PP, DC, 128], BF16,
                                           tag="ptr", name=f"ptr{p}")
                    attTs[p] = attpool.tile([128, N_PP, C], BF16,
                                            tag="attT", name=f"attT{p}")
                    tsm = attpool.tile([128, BPC, N_PP], BF16, tag="tsm",
                                       name=f"tsm{p}")
                    nc.vector.memset(tsm[:], 0.0)
                    tsms[p] = tsm
                nat = nats[p]
                ps_tr = trpsum.tile([128, D], BF16, tag="tr",
                                    name=f"tr_{p}_{n}")
                for j in range(DC):
                    nc.tensor.transpose(
                        ps_tr[:, j * 128:(j + 1) * 128],
                        nat[:, n, j * 128:(j + 1) * 128],
                        ident_b[:])
                if n % 2 == 0:
                    nc.vector.tensor_copy(ptrs[p][:, n, :, :], ps_tr[:])
                else:
                    nc.scalar.copy(ptrs[p][:, n, :, :], ps_tr[:])

            def emit_e1(p, n):
                ps_l = lpsum.tile([128, C + 1], F32, tag="l")
                for dc in range(DC):
                    nc.tensor.matmul(ps_l[:], ptrs[p][:, n, dc, :],
                                     waug[:, dc, :],
                                     start=(dc == 0), stop=(dc == DC - 1))
                nc.vector.tensor_add(ps_l[:, 0:C], ps_l[:, 0:C], bias_bc[:])
                nc.scalar.activation(attTs[p][:, n, :], ps_l[:, 0:C],
                                     AF.Sigmoid)
                # lam-scaled token sums -> masked ts-matrix columns
                tsm = tsms[p]
                nc.vector.tensor_copy(tsm[0:64, 2 * p, n:n + 1],
                                      ps_l[0:64, C:C + 1])
                nc.vector.tensor_copy(tsm[64:128, 2 * p + 1, n:n + 1],
                                      ps_l[64:128, C:C + 1])

            def emit_e2(p):
                if not ps_a2_box:
                    ps_a2_box.append(spsum.tile([BPC, C], F32, tag="sp",
                                                name="ps_a2"))
                ps_a2 = ps_a2_box[0]
                attT, tsm = attTs.pop(p), tsms.pop(p)
                del ptrs[p]
                for n in range(N_PP):
                    nc.tensor.matmul(ps_a2[:],
                                     tsm[:, :, n],
                                     attT[:, n, :],
                                     start=(p == 0 and n == 0),
                                     stop=(p == PAIRS - 1 and n == N_PP - 1))

            TOT = PAIRS * N_PP
            for k in range(TOT):
                p, n = divmod(k, N_PP)
                emit_tr(p, n)
                if k == 12:
                    emit_gs_setup()
                if k >= 2:
                    emit_e1(*divmod(k - 2, N_PP))
                if n == 3 and p >= 1:
                    emit_e2(p - 1)
            emit_e1(PAIRS - 1, N_PP - 2)
            emit_e1(PAIRS - 1, N_PP - 1)
            emit_e2(PAIRS - 1)

            # ---------------- final combine + store ----------------
            nc.vector.tensor_add(outsb[:], ps_a2_box[0][:], gs_sb[:])
            nc.sync.dma_start(out[:], outsb[:])

    nc.compile()
    return nc


def _get_compiled():
    global _COMPILED
    if _COMPILED is None:
        _COMPILED = _build()
    return _COMPILED


def kernel(patch_tokens, class_token, attn_w, attn_b, gc_w, gc_b, lam,
           **_ignored):
    nc = _get_compiled()
    patch_tokens = np.ascontiguousarray(patch_tokens, dtype=np.float32)
    class_token = np.ascontiguousarray(class_token, dtype=np.float32)
    attn_w = np.ascontiguousarray(attn_w, dtype=np.float32)
    attn_b = np.ascontiguousarray(attn_b, dtype=np.float32)
    gc_w = np.ascontiguousarray(gc_w, dtype=np.float32)
    gc_b = np.ascontiguousarray(gc_b, dtype=np.float32)
    lam = np.ascontiguousarray(lam, dtype=np.float32)

    in_maps = []
    for i in range(NCORES):
        sl = slice(i * BPC, (i + 1) * BPC)
        in_maps.append({
            "pt": patch_tokens[sl],
            "ct": class_token[sl],
            "aw": attn_w,
            "ab": attn_b,
            "gw": gc_w,
            "gb": gc_b,
            "lam": lam,
        })
    res = run_bass_kernel_spmd(nc, in_maps, core_ids=list(range(NCORES)))
    return np.concatenate([res.results[i]["out"] for i in range(NCORES)],
                          axis=0)
